# revision 16
# baseline (speedup 1.0000x reference)
"""MultiLabelMarginLoss kernel for Trainium2, data-parallel over 8 cores — v3.

Reference semantics (B=64, C=1536):
    loss = mean_i [ sum_{p in pos_i, n in neg_i} relu(1 - x_p + x_n) / (|pos_i| * |neg_i|) ]
pos_i = distinct class indices listed before the first -1 in target[i].

v3 redesign (driven by the instruction cost model):
  * Host packs each core's positives ("slots") tightly across samples into
    NBLK blocks of 128 partition slots (NBLK = ceil(max core positives /128),
    data-adaptive; samples are LPT-balanced across cores by positive count).
  * One broadcast matmul per 512-col chunk: stationary column p selects the
    slot's sample row AND a mask row (-BIG at that sample's positive classes),
    so out[p, c] = x_{s(p),c} + mask_{s(p),c}.  Masked classes relu to zero,
    eliminating the baseline's separate positive-vs-positive correction pass.
  * Bias 1 - x_p rides the ScalarE activation / DVE custom-op per-partition
    scalar operand; the host supplies it with the packed metadata so nothing
    gates the main phase except the two input DMAs.
  * relu+sum fused ops split between ScalarE (wide units) and VectorE
    (512-wide units), balanced by modeled cost; per-slot accumulators
    [128, n_units] are DMA'd out raw and the host applies the 1/(k(C-k)B)
    weights and the final sum (the scalar all-reduce).
  * Everything ships in two DMAs: `big` ([16, C+CAP] bf16: pred rows 0-7,
    mask rows 8-15, selector columns appended) and `meta` ([128, NBLK] f32
    bias).  bf16 halves DMA bytes and keeps the matmul at 1 cycle/col with
    no f32r small-tile penalties; PSUM accumulation stays fp32.
"""

import numpy as np
from contextlib import ExitStack

import concourse.bass as bass
import concourse.tile as tile
import concourse.dve_ops as dve_ops
from concourse import bacc, mybir
from concourse.bass_utils import run_bass_kernel_spmd
from concourse.dve_spec import Spec, Src0, C0, relu, lower
from concourse.dve_uop import DveOpSpec
from operator import add as _op_add


def _get_relu_bias_sum_op():
    """Custom DVE op: out = relu(in0 + s0); accum_out = sum(out, free axis)."""
    name = "RELU_BIAS_SUM_MLML"
    for op in dve_ops.OPS:
        if op.name == name:
            return op

    def _ref(in0, in1, c0, c1, c2):
        b = np.maximum(in0.astype(np.float32) + c0, 0.0).astype(np.float32)
        return b, b.reshape(b.shape[0], -1).sum(axis=-1, keepdims=True)

    spec = Spec(body=relu(Src0 + C0), accum=_op_add, reference=_ref)
    op = dve_ops.DveOp(name, spec, subdim=False, uops_sha={})
    row = dve_ops._CUSTOM_DVE_ROW_BASE + len(dve_ops.OPS)
    assert row < 0x20
    dve_ops.OPS.append(op)
    dve_ops.CUSTOM_DVE_SPECS[name] = spec
    dve_ops._SUB_OPCODE_FOR_NAME[name] = row
    for ver in ("v3", "v4"):
        compiled = DveOpSpec(
            name=name,
            opcode=row,
            uops=lower(spec, ver=ver),
            rd1_en=False,
        )
        op.uops_sha[ver] = compiled.sha(ver)
    return op


B, C = 64, 1536
M = 8            # cores
BL = B // M      # samples per core
BIG = 1.0e9
FP32 = mybir.dt.float32
BF16 = mybir.dt.bfloat16
CHUNK = 512

# per-unit engine cost (ns) used to balance the ScalarE / VectorE lanes
def _act_ns(w):
    return 0.833 * w + 372.0


def _dve_ns(w):
    return 1.042 * w + 125.0


def _block0_chunks(am):
    """DVE chunks of block 0 covering [am, 1536), none crossing a 512-grid
    bank boundary, smallest chunk last in column order (it is emitted first
    so the DVE lane starts after the shortest possible matmul)."""
    chunks = []
    lo = am
    while lo < 1536:
        hi = min((lo // CHUNK + 1) * CHUNK, 1536)
        chunks.append((lo, hi))
        lo = hi
    # move the smallest chunk to the highest columns by construction: the
    # only sub-512 chunk is the first (grid remainder); emission order below
    # is by descending lo, so keep as-is and emit accordingly.
    return chunks


def _lane_plan(nblk):
    """Return the ordered unit list [(lane, block, lo, hi)].

    Block 0 is a mixed block: a small leading DVE unit (short matmul, so the
    DVE lane starts earliest), then the rest of its DVE columns, then its ACT
    portion.  Remaining blocks alternate full-DVE (3x512 units) and full-ACT
    (one 1536-wide unit), with the full-ACT count and block-0 split chosen by
    the modeled lane-finish balance (ACT's lane starts ~0.6us later)."""
    ACT_OFF = 612.0
    if nblk == 5:
        # timeline-sim-measured optimum for the common shape
        best = (0.0, 2, 896)
    else:
        best = None
    for n_act in range(nblk if best is None else 0):
        for am in range(0, 1537, 128):
            if am == 0 and n_act == 0:
                continue
            act = ((_act_ns(am) if am else 0.0)
                   + n_act * _act_ns(1536))
            dve = sum(_dve_ns(hi - lo) for lo, hi in _block0_chunks(am))
            dve += (nblk - 1 - n_act) * 3 * _dve_ns(CHUNK)
            m = max(act + (ACT_OFF if act else 0.0), dve)
            if best is None or m < best[0]:
                best = (m, n_act, am)
    _, n_act, am = best

    units = []
    # block 0: DVE chunks in descending lo (smallest/grid-remainder chunk is
    # at the highest columns only when am is 512-aligned; emit smallest first)
    b0 = sorted(_block0_chunks(am), key=lambda c: c[1] - c[0])
    units.extend([("D", 0, lo, hi) for lo, hi in b0])
    if am:
        units.append(("A", 0, 0, am))
    n_dve = nblk - 1 - n_act
    order = []
    a_left, d_left = n_act, n_dve
    while d_left or a_left:
        if d_left:
            order.append("D")
            d_left -= 1
        if a_left:
            order.append("A")
            a_left -= 1
    for i, kind in enumerate(order):
        b = 1 + i
        if kind == "A":
            units.append(("A", b, 0, 1536))
        else:
            units.extend([("D", b, q * CHUNK, (q + 1) * CHUNK) for q in range(3)])
    return units


def _build_nc(nblk, warm_pe=False):
    RELU_BIAS_SUM = _get_relu_bias_sum_op()
    RELU = mybir.ActivationFunctionType.Relu
    cap = nblk * 128
    W = C + cap  # big free width

    units = _lane_plan(nblk)  # ordered (lane, block, lo, hi)
    nu = len(units)
    max_aw = max([u[3] - u[2] for u in units if u[0] == "A"], default=CHUNK)
    max_dw = max([u[3] - u[2] for u in units if u[0] == "D"], default=CHUNK)
    assert 2 * (max_aw + max_dw) <= 4096, "PSUM budget exceeded"

    nc = bacc.Bacc("TRN2", target_bir_lowering=False, debug=False, num_devices=M)
    big_d = nc.dram_tensor("big", [16, W], BF16, kind="ExternalInput")
    meta_d = nc.dram_tensor("meta", [128, nblk], FP32, kind="ExternalInput")
    acc_d = nc.dram_tensor("acc", [128, nu], FP32, kind="ExternalOutput")

    with tile.TileContext(nc) as tc, ExitStack() as ctx:
        const = ctx.enter_context(tc.tile_pool(name="const", bufs=1))
        sbuf = ctx.enter_context(tc.tile_pool(name="sbuf", bufs=1))
        scratch = ctx.enter_context(tc.tile_pool(name="scratch", bufs=2))
        psA = ctx.enter_context(tc.tile_pool(name="psA", bufs=2, space="PSUM"))
        psB = ctx.enter_context(tc.tile_pool(name="psB", bufs=2, space="PSUM"))

        big_sb = const.tile([16, W], BF16)
        nc.sync.dma_start(big_sb[:], big_d.ap())
        # meta rides the (otherwise idle) Pool SWDGE path so it never queues
        # behind `big` on the shared HWDGE
        bias_t = const.tile([128, nblk], FP32)
        nc.gpsimd.dma_start(bias_t[:], meta_d.ap())

        # warm the ACT function table before the first real activation
        warm = const.tile([128, 1], FP32)
        nc.vector.memset(warm[:], 1.0)
        warm2 = const.tile([128, 1], FP32)
        nc.scalar.activation(warm2[:], warm[:], RELU)

        if warm_pe:
            # dummy matmuls start the PE p-state ramp while the input DMAs land
            wsrc = const.tile([16, CHUNK], BF16)
            nc.gpsimd.memset(wsrc[:], 0.0)
            for _ in range(4):
                wps = psB.tile([128, CHUNK], FP32, tag="B")
                nc.tensor.matmul(
                    wps[:], lhsT=wsrc[:, :128], rhs=wsrc[:], start=True, stop=True
                )

        acc = sbuf.tile([128, nu], FP32)
        for ui, (lane, b, lo, hi) in enumerate(units):
            sel = big_sb[:, C + b * 128:C + (b + 1) * 128]
            bias_s = bias_t[:, b:b + 1]
            wcols = hi - lo
            if lane == "A":
                ps = psA.tile([128, wcols], FP32, tag="A")
                for off in range(0, wcols, CHUNK):
                    end = min(off + CHUNK, wcols)
                    nc.tensor.matmul(
                        ps[:, off:end],
                        lhsT=sel,
                        rhs=big_sb[:, lo + off:lo + end],
                        start=True, stop=True,
                    )
                scr = scratch.tile([128, max_aw], FP32, tag="scrA")
                nc.scalar.activation(
                    scr[:, :wcols], ps[:], RELU, bias=bias_s, scale=1.0,
                    accum_out=acc[:, ui:ui + 1],
                )
            else:
                ps = psB.tile([128, max_dw], FP32, tag="B")
                for off in range(0, wcols, CHUNK):
                    end = min(off + CHUNK, wcols)
                    nc.tensor.matmul(
                        ps[:, off:end], lhsT=sel,
                        rhs=big_sb[:, lo + off:lo + end],
                        start=True, stop=True,
                    )
                scr = scratch.tile([128, max_dw], FP32, tag="scrB")
                nc.vector._custom_dve(
                    RELU_BIAS_SUM,
                    out=scr[:, :wcols], in0=ps[:, :wcols], s0=bias_s,
                    accum_out=acc[:, ui:ui + 1],
                )

        nc.sync.dma_start(acc_d.ap(), acc[:])

    nc.compile()
    nc._mlml_units = units
    return nc


_NCS = {}


def _get_nc(nblk):
    if nblk not in _NCS:
        _NCS[nblk] = _build_nc(nblk)
    return _NCS[nblk]


def _plan(pred, tgt):
    """Host-side packing of target metadata.  Returns (nblk, per-core input
    dicts, per-core unit weight matrices, per-core float64 reference
    partials)."""
    import ml_dtypes

    pred = np.ascontiguousarray(np.asarray(pred), dtype=np.float32)
    tgt = np.asarray(tgt)
    b, c = pred.shape
    assert (b, c) == (B, C)

    # distinct positives per sample (entries before first -1)
    pos_lists = []
    ks = np.zeros(B, np.int64)
    for s in range(B):
        t = np.asarray(tgt[s]).astype(np.int64)
        valid = np.cumprod(t != -1).astype(bool)
        pos = np.unique(t[valid])
        pos_lists.append(pos)
        ks[s] = len(pos)

    # LPT-balance samples across cores by positive count (8 samples per core)
    order = np.argsort(-ks, kind="stable")
    loads = [0] * M
    counts = [0] * M
    assign = [[] for _ in range(M)]
    for i in order:
        for cc in sorted(range(M), key=lambda x: (loads[x], x)):
            if counts[cc] < BL:
                assign[cc].append(int(i))
                loads[cc] += int(ks[i])
                counts[cc] += 1
                break
    nblk = min(8, max(1, -(-max(loads) // 128)))
    cap = nblk * 128
    W = C + cap

    nc = _get_nc(nblk)
    units = nc._mlml_units
    ublock = np.array([u[1] for u in units], np.int64)

    bf = ml_dtypes.bfloat16
    in_maps, weights = [], []
    for core in range(M):
        big = np.zeros((16, W), np.float32)
        bias = np.zeros((128, nblk), np.float32)
        wslot = np.zeros((128, nblk), np.float32)
        p = 0
        for sl, s in enumerate(assign[core]):
            big[sl, :C] = pred[s]
            pos = pos_lists[s]
            k = len(pos)
            if k:
                big[8 + sl, pos] = -BIG
            if k == 0 or k == C:
                continue
            w = 1.0 / (float(k) * float(C - k) * float(B))
            for cls in pos:
                blk, slot = divmod(p, 128)
                big[sl, C + blk * 128 + slot] = 1.0
                big[8 + sl, C + blk * 128 + slot] = 1.0
                bias[slot, blk] = 1.0 - pred[s, cls]
                wslot[slot, blk] = w
                p += 1
        assert p <= cap
        in_maps.append({
            "big": np.ascontiguousarray(big.astype(bf)),
            "meta": np.ascontiguousarray(bias),
        })
        weights.append(np.ascontiguousarray(wslot[:, ublock]))

    # float64 reference partial per core (for testing/debug only)
    partials = []
    for core in range(M):
        tot = 0.0
        for s in assign[core]:
            pos = pos_lists[s]
            k = len(pos)
            if k == 0 or k == C:
                continue
            x = pred[s].astype(np.float64)
            xp = x[pos]
            neg = np.ones(C, bool)
            neg[pos] = False
            xn = x[neg]
            m = np.maximum(1.0 - xp[:, None] + xn[None, :], 0.0).sum()
            tot += m / (k * (C - k)) / B
        partials.append(tot)
    return nblk, in_maps, weights, partials


def kernel(pred, target):
    nblk, in_maps, weights, _ = _plan(pred, target)
    nc = _get_nc(nblk)
    res = run_bass_kernel_spmd(nc, in_maps, core_ids=list(range(M)))
    total = 0.0
    for core in range(M):
        acc = np.asarray(res.results[core]["acc"], dtype=np.float64)
        total += float((acc * weights[core]).sum())
    return np.asarray(total, dtype=np.float32)


# revision 17
# speedup vs baseline: 1.0015x; 1.0015x over previous
"""MultiLabelMarginLoss kernel for Trainium2, data-parallel over 8 cores — v3.

Reference semantics (B=64, C=1536):
    loss = mean_i [ sum_{p in pos_i, n in neg_i} relu(1 - x_p + x_n) / (|pos_i| * |neg_i|) ]
pos_i = distinct class indices listed before the first -1 in target[i].

v3 redesign (driven by the instruction cost model):
  * Host packs each core's positives ("slots") tightly across samples into
    NBLK blocks of 128 partition slots (NBLK = ceil(max core positives /128),
    data-adaptive; samples are LPT-balanced across cores by positive count).
  * One broadcast matmul per 512-col chunk: stationary column p selects the
    slot's sample row AND a mask row (-BIG at that sample's positive classes),
    so out[p, c] = x_{s(p),c} + mask_{s(p),c}.  Masked classes relu to zero,
    eliminating the baseline's separate positive-vs-positive correction pass.
  * Bias 1 - x_p rides the ScalarE activation / DVE custom-op per-partition
    scalar operand; the host supplies it with the packed metadata so nothing
    gates the main phase except the two input DMAs.
  * relu+sum fused ops split between ScalarE (wide units) and VectorE
    (512-wide units), balanced by modeled cost; per-slot accumulators
    [128, n_units] are DMA'd out raw and the host applies the 1/(k(C-k)B)
    weights and the final sum (the scalar all-reduce).
  * Everything ships in two DMAs: `big` ([16, C+CAP] bf16: pred rows 0-7,
    mask rows 8-15, selector columns appended) and `meta` ([128, NBLK] f32
    bias).  bf16 halves DMA bytes and keeps the matmul at 1 cycle/col with
    no f32r small-tile penalties; PSUM accumulation stays fp32.
"""

import numpy as np
from contextlib import ExitStack

import concourse.bass as bass
import concourse.tile as tile
import concourse.dve_ops as dve_ops
from concourse import bacc, mybir
from concourse.bass_utils import run_bass_kernel_spmd
from concourse.dve_spec import Spec, Src0, C0, relu, lower
from concourse.dve_uop import DveOpSpec
from operator import add as _op_add


def _get_relu_bias_sum_op():
    """Custom DVE op: out = relu(in0 + s0); accum_out = sum(out, free axis)."""
    name = "RELU_BIAS_SUM_MLML"
    for op in dve_ops.OPS:
        if op.name == name:
            return op

    def _ref(in0, in1, c0, c1, c2):
        b = np.maximum(in0.astype(np.float32) + c0, 0.0).astype(np.float32)
        return b, b.reshape(b.shape[0], -1).sum(axis=-1, keepdims=True)

    spec = Spec(body=relu(Src0 + C0), accum=_op_add, reference=_ref)
    op = dve_ops.DveOp(name, spec, subdim=False, uops_sha={})
    row = dve_ops._CUSTOM_DVE_ROW_BASE + len(dve_ops.OPS)
    assert row < 0x20
    dve_ops.OPS.append(op)
    dve_ops.CUSTOM_DVE_SPECS[name] = spec
    dve_ops._SUB_OPCODE_FOR_NAME[name] = row
    for ver in ("v3", "v4"):
        compiled = DveOpSpec(
            name=name,
            opcode=row,
            uops=lower(spec, ver=ver),
            rd1_en=False,
        )
        op.uops_sha[ver] = compiled.sha(ver)
    return op


B, C = 64, 1536
M = 8            # cores
BL = B // M      # samples per core
BIG = 1.0e9
FP32 = mybir.dt.float32
BF16 = mybir.dt.bfloat16
CHUNK = 512

# per-unit engine cost (ns) used to balance the ScalarE / VectorE lanes
def _act_ns(w):
    return 0.833 * w + 372.0


def _dve_ns(w):
    return 1.042 * w + 125.0


def _block0_chunks(am):
    """DVE chunks of block 0 covering [am, 1536), none crossing a 512-grid
    bank boundary, smallest chunk last in column order (it is emitted first
    so the DVE lane starts after the shortest possible matmul)."""
    chunks = []
    lo = am
    while lo < 1536:
        hi = min((lo // CHUNK + 1) * CHUNK, 1536)
        chunks.append((lo, hi))
        lo = hi
    # move the smallest chunk to the highest columns by construction: the
    # only sub-512 chunk is the first (grid remainder); emission order below
    # is by descending lo, so keep as-is and emit accordingly.
    return chunks


def _lane_plan(nblk):
    """Return the ordered unit list [(lane, block, lo, hi)].

    Block 0 is a mixed block: a small leading DVE unit (short matmul, so the
    DVE lane starts earliest), then the rest of its DVE columns, then its ACT
    portion.  Remaining blocks alternate full-DVE (3x512 units) and full-ACT
    (one 1536-wide unit), with the full-ACT count and block-0 split chosen by
    the modeled lane-finish balance (ACT's lane starts ~0.6us later)."""
    ACT_OFF = 612.0
    if nblk == 5:
        # timeline-sim-measured optimum for the common shape
        best = (0.0, 2, 896)
    else:
        best = None
    for n_act in range(nblk if best is None else 0):
        for am in range(0, 1537, 128):
            if am == 0 and n_act == 0:
                continue
            act = ((_act_ns(am) if am else 0.0)
                   + n_act * _act_ns(1536))
            dve = sum(_dve_ns(hi - lo) for lo, hi in _block0_chunks(am))
            dve += (nblk - 1 - n_act) * 3 * _dve_ns(CHUNK)
            m = max(act + (ACT_OFF if act else 0.0), dve)
            if best is None or m < best[0]:
                best = (m, n_act, am)
    _, n_act, am = best

    units = []
    # block 0: DVE chunks in descending lo (smallest/grid-remainder chunk is
    # at the highest columns only when am is 512-aligned; emit smallest first)
    b0 = sorted(_block0_chunks(am), key=lambda c: c[1] - c[0])
    units.extend([("D", 0, lo, hi) for lo, hi in b0])
    if am:
        units.append(("A", 0, 0, am))
    n_dve = nblk - 1 - n_act
    order = []
    a_left, d_left = n_act, n_dve
    while d_left or a_left:
        if d_left:
            order.append("D")
            d_left -= 1
        if a_left:
            order.append("A")
            a_left -= 1
    for i, kind in enumerate(order):
        b = 1 + i
        if kind == "A":
            units.append(("A", b, 0, 1536))
        else:
            units.extend([("D", b, q * CHUNK, (q + 1) * CHUNK) for q in range(3)])
    return units


def _build_nc(nblk, warm_pe=False):
    RELU_BIAS_SUM = _get_relu_bias_sum_op()
    RELU = mybir.ActivationFunctionType.Relu
    cap = nblk * 128
    W = C + cap  # big free width

    units = _lane_plan(nblk)  # ordered (lane, block, lo, hi)
    nu = len(units)
    max_aw = max([u[3] - u[2] for u in units if u[0] == "A"], default=CHUNK)
    max_dw = max([u[3] - u[2] for u in units if u[0] == "D"], default=CHUNK)
    assert 2 * (max_aw + max_dw) <= 4096, "PSUM budget exceeded"

    nc = bacc.Bacc("TRN2", target_bir_lowering=False, debug=False, num_devices=M)
    big_d = nc.dram_tensor("big", [16, W], BF16, kind="ExternalInput")
    meta_d = nc.dram_tensor("meta", [128, nblk], FP32, kind="ExternalInput")
    acc_d = nc.dram_tensor("acc", [128, nu], FP32, kind="ExternalOutput")

    with tile.TileContext(nc) as tc, ExitStack() as ctx:
        const = ctx.enter_context(tc.tile_pool(name="const", bufs=1))
        sbuf = ctx.enter_context(tc.tile_pool(name="sbuf", bufs=1))
        scratch = ctx.enter_context(tc.tile_pool(name="scratch", bufs=2))
        psA = ctx.enter_context(tc.tile_pool(name="psA", bufs=2, space="PSUM"))
        psB = ctx.enter_context(tc.tile_pool(name="psB", bufs=2, space="PSUM"))

        big_sb = const.tile([16, W], BF16)
        nc.sync.dma_start(big_sb[:], big_d.ap())
        # meta rides the (otherwise idle) Pool SWDGE path so it never queues
        # behind `big` on the shared HWDGE
        bias_t = const.tile([128, nblk], FP32)
        nc.gpsimd.dma_start(bias_t[:], meta_d.ap())

        # warm the ACT function table before the first real activation
        warm = const.tile([128, 1], FP32)
        nc.vector.memset(warm[:], 1.0)
        warm2 = const.tile([128, 1], FP32)
        nc.scalar.activation(warm2[:], warm[:], RELU)

        if warm_pe:
            # dummy matmuls start the PE p-state ramp while the input DMAs land
            wsrc = const.tile([16, CHUNK], BF16)
            nc.gpsimd.memset(wsrc[:], 0.0)
            for _ in range(4):
                wps = psB.tile([128, CHUNK], FP32, tag="B")
                nc.tensor.matmul(
                    wps[:], lhsT=wsrc[:, :128], rhs=wsrc[:], start=True, stop=True
                )

        acc = sbuf.tile([128, nu], FP32)
        first_a = next((i for i, u in enumerate(units) if u[0] == "A"), None)
        for ui, (lane, b, lo, hi) in enumerate(units):
            sel = big_sb[:, C + b * 128:C + (b + 1) * 128]
            bias_s = bias_t[:, b:b + 1]
            wcols = hi - lo
            if lane == "A":
                ps = psA.tile([128, wcols], FP32, tag="A")
                for off in range(0, wcols, CHUNK):
                    end = min(off + CHUNK, wcols)
                    nc.tensor.matmul(
                        ps[:, off:end],
                        lhsT=sel,
                        rhs=big_sb[:, lo + off:lo + end],
                        start=True, stop=True,
                    )
                if ui == first_a:
                    # Pool is idle after the meta DMA: offload this unit's
                    # free-axis sum to it (from SBUF, which Pool can read) so
                    # ScalarE skips the serializing accum-read aux op.
                    scr0 = const.tile([128, wcols], FP32)
                    nc.scalar.activation(
                        scr0[:], ps[:], RELU, bias=bias_s, scale=1.0,
                    )
                    pscr = const.tile([128, wcols], FP32)
                    nc.gpsimd.tensor_scalar(
                        pscr[:], scr0[:], 0.0, None, op0=mybir.AluOpType.add,
                        accum_out=acc[:, ui:ui + 1],
                    )
                    continue
                scr = scratch.tile([128, max_aw], FP32, tag="scrA")
                nc.scalar.activation(
                    scr[:, :wcols], ps[:], RELU, bias=bias_s, scale=1.0,
                    accum_out=acc[:, ui:ui + 1],
                )
            else:
                ps = psB.tile([128, max_dw], FP32, tag="B")
                for off in range(0, wcols, CHUNK):
                    end = min(off + CHUNK, wcols)
                    nc.tensor.matmul(
                        ps[:, off:end], lhsT=sel,
                        rhs=big_sb[:, lo + off:lo + end],
                        start=True, stop=True,
                    )
                scr = scratch.tile([128, max_dw], FP32, tag="scrB")
                nc.vector._custom_dve(
                    RELU_BIAS_SUM,
                    out=scr[:, :wcols], in0=ps[:, :wcols], s0=bias_s,
                    accum_out=acc[:, ui:ui + 1],
                )

        nc.sync.dma_start(acc_d.ap(), acc[:])

    nc.compile()
    nc._mlml_units = units
    return nc


_NCS = {}


def _get_nc(nblk):
    if nblk not in _NCS:
        _NCS[nblk] = _build_nc(nblk)
    return _NCS[nblk]


def _plan(pred, tgt):
    """Host-side packing of target metadata.  Returns (nblk, per-core input
    dicts, per-core unit weight matrices, per-core float64 reference
    partials)."""
    import ml_dtypes

    pred = np.ascontiguousarray(np.asarray(pred), dtype=np.float32)
    tgt = np.asarray(tgt)
    b, c = pred.shape
    assert (b, c) == (B, C)

    # distinct positives per sample (entries before first -1)
    pos_lists = []
    ks = np.zeros(B, np.int64)
    for s in range(B):
        t = np.asarray(tgt[s]).astype(np.int64)
        valid = np.cumprod(t != -1).astype(bool)
        pos = np.unique(t[valid])
        pos_lists.append(pos)
        ks[s] = len(pos)

    # LPT-balance samples across cores by positive count (8 samples per core)
    order = np.argsort(-ks, kind="stable")
    loads = [0] * M
    counts = [0] * M
    assign = [[] for _ in range(M)]
    for i in order:
        for cc in sorted(range(M), key=lambda x: (loads[x], x)):
            if counts[cc] < BL:
                assign[cc].append(int(i))
                loads[cc] += int(ks[i])
                counts[cc] += 1
                break
    nblk = min(8, max(1, -(-max(loads) // 128)))
    cap = nblk * 128
    W = C + cap

    nc = _get_nc(nblk)
    units = nc._mlml_units
    ublock = np.array([u[1] for u in units], np.int64)

    bf = ml_dtypes.bfloat16
    in_maps, weights = [], []
    for core in range(M):
        big = np.zeros((16, W), np.float32)
        bias = np.zeros((128, nblk), np.float32)
        wslot = np.zeros((128, nblk), np.float32)
        p = 0
        for sl, s in enumerate(assign[core]):
            big[sl, :C] = pred[s]
            pos = pos_lists[s]
            k = len(pos)
            if k:
                big[8 + sl, pos] = -BIG
            if k == 0 or k == C:
                continue
            w = 1.0 / (float(k) * float(C - k) * float(B))
            for cls in pos:
                blk, slot = divmod(p, 128)
                big[sl, C + blk * 128 + slot] = 1.0
                big[8 + sl, C + blk * 128 + slot] = 1.0
                bias[slot, blk] = 1.0 - pred[s, cls]
                wslot[slot, blk] = w
                p += 1
        assert p <= cap
        in_maps.append({
            "big": np.ascontiguousarray(big.astype(bf)),
            "meta": np.ascontiguousarray(bias),
        })
        weights.append(np.ascontiguousarray(wslot[:, ublock]))

    # float64 reference partial per core (for testing/debug only)
    partials = []
    for core in range(M):
        tot = 0.0
        for s in assign[core]:
            pos = pos_lists[s]
            k = len(pos)
            if k == 0 or k == C:
                continue
            x = pred[s].astype(np.float64)
            xp = x[pos]
            neg = np.ones(C, bool)
            neg[pos] = False
            xn = x[neg]
            m = np.maximum(1.0 - xp[:, None] + xn[None, :], 0.0).sum()
            tot += m / (k * (C - k)) / B
        partials.append(tot)
    return nblk, in_maps, weights, partials


def kernel(pred, target):
    nblk, in_maps, weights, _ = _plan(pred, target)
    nc = _get_nc(nblk)
    res = run_bass_kernel_spmd(nc, in_maps, core_ids=list(range(M)))
    total = 0.0
    for core in range(M):
        acc = np.asarray(res.results[core]["acc"], dtype=np.float64)
        total += float((acc * weights[core]).sum())
    return np.asarray(total, dtype=np.float32)
